# revision 43
# baseline (speedup 1.0000x reference)
"""Trainium2 Bass kernel for nn_CONCATNet_7447473291796 (gnn_message_passing).

Strategy (pure data parallelism, batch sharded 16 per core across 8 cores):
  The reference only ever *uses* ~66 of the 4096 wafer rows per batch. The
  host gathers exactly those rows (plus the stage / next-stage / arm rows)
  while sharding the batch, and hands each core dense, pre-transposed bf16
  tiles with the embed dim on partitions:

    xcolA [128, 832]   w_cs | w_cw | stage rows pm 0..511 | arm-loc | next-stage
    xrowA [128, 576]   wafer rows pm 0..511 | arm-loc | arm-recipe
    xcolB/xrowB [128, 512]  pm columns 512..1023

  The whole module is linear, so every non-gather term is folded on host:
  - robot-arm head -> fused weights (W_cs@W_rl, W_cw@W_rl)
  - the pm_dyn rank-1 term and the loc==P+1 ones-row -> a min-norm solve
    of [W_cs.T | W_cw.T] z = rhs (sigma_min ~0.23, so no noise blow-up)
    adds rp*u / rp*v (and ind*u2 / ind*v2) into the gathered rows.

  The device is then just 8 matmuls in a transposed layout out[d_out, rows]:
    pmT = W_cs.T @ xcol + W_cw.T @ xrow          (N=512, two tiles)
    armT = fused(W)s over the 64 arm columns      (N=32, four matmuls)

  Hand-scheduled raw bass (no TileContext): per-DMA semaphores, engines
  free-run with minimal waits.  bf16 in/out with fp32 PSUM accumulation
  keeps rel err ~4e-3 (gate is 2e-2).

All per-core variation lives in the DRAM inputs; the Bass program is
identical on every core.
"""

import numpy as np
import ml_dtypes

import concourse.bass as bass
import concourse.bacc as bacc
import concourse.mybir as mybir
from concourse.bass_utils import run_bass_kernel_spmd

B, N, S, P, D = 128, 4096, 32, 64, 128
NORM = 300.0
NCORES = 8
BL = B // NCORES          # local batches per core = 16
R = BL * P                # pm columns per core = 1024
A = 2 * BL                # arm columns per core = 32
H = R // 2                # pm columns per tile = 512
WA = 2 * D                # w_cs|w_cw packed at the head of xcolA

F32 = mybir.dt.float32
BF16 = mybir.dt.bfloat16
BF = ml_dtypes.bfloat16

_prog_cache = None


def _build_program():
    nc = bacc.Bacc("TRN2", target_bir_lowering=False, debug=False)

    wA_h = nc.declare_dram_parameter("wA", [128, WA], BF16, isOutput=False)
    xcolA_h = nc.declare_dram_parameter("xcolA", [128, H + 2 * A], BF16,
                                        isOutput=False)
    xcolB_h = nc.declare_dram_parameter("xcolB", [128, H], BF16, isOutput=False)
    xrowA_h = nc.declare_dram_parameter("xrowA", [128, H + 2 * A], BF16, isOutput=False)
    xrowB_h = nc.declare_dram_parameter("xrowB", [128, H], BF16, isOutput=False)
    wB_h = nc.declare_dram_parameter("wB", [128, 4, D], BF16, isOutput=False)

    out0a_h = nc.declare_dram_parameter("out0a", [128, H + A], BF16, isOutput=True)
    out1_h = nc.declare_dram_parameter("out1", [128, H], BF16, isOutput=True)

    from contextlib import ExitStack
    with ExitStack() as stack:
        ec = stack.enter_context
        wAsb = ec(nc.sbuf_tensor([128, WA], BF16))
        xcolA = ec(nc.sbuf_tensor([128, H + 2 * A], BF16))
        xcolB = ec(nc.sbuf_tensor([128, H], BF16))
        xrowA = ec(nc.sbuf_tensor([128, H + 2 * A], BF16))
        xrowB = ec(nc.sbuf_tensor([128, H], BF16))
        wBsb = ec(nc.sbuf_tensor([128, 4, D], BF16))
        o0a = ec(nc.sbuf_tensor([128, H + A], BF16))
        o1 = ec(nc.sbuf_tensor([128, H], BF16))
        ps0 = ec(nc.psum_tensor([128, H], F32))
        ps1 = ec(nc.psum_tensor([128, H], F32))
        psr = ec(nc.psum_tensor([128, A], F32))
        s_wA = ec(nc.semaphore("s_wA"))
        s_xcA = ec(nc.semaphore("s_xcA"))
        s_xcB = ec(nc.semaphore("s_xcB"))
        s_xrA = ec(nc.semaphore("s_xrA"))
        s_xrB = ec(nc.semaphore("s_xrB"))
        s_wB = ec(nc.semaphore("s_wB"))
        s_st0 = ec(nc.semaphore("s_st0"))
        s_st1 = ec(nc.semaphore("s_st1"))
        t0 = ec(nc.semaphore("t0"))
        t1 = ec(nc.semaphore("t1"))
        t2 = ec(nc.semaphore("t2"))
        v0 = ec(nc.semaphore("v0"))
        v1 = ec(nc.semaphore("v1"))
        c0 = ec(nc.semaphore("c0"))
        block = ec(nc.Block())

        w_cs = wAsb[:, 0:D]
        w_cw = wAsb[:, D : 2 * D]
        w_rw = wBsb[:, 0, :]
        w_rn = wBsb[:, 1, :]
        w_fcs = wBsb[:, 2, :]    # W_cs @ W_rl
        w_fcw = wBsb[:, 3, :]    # W_cw @ W_rl
        xcA = xcolA[:, 0:H]                      # stage pm 0..511
        xcA_loc = xcolA[:, H : H + A]            # arm-loc stage (folded)
        xcA_ns = xcolA[:, H + A : H + 2 * A]

        @block.sync
        def _(sync):
            sync.dma_start(xcolA[:], xcolA_h[:]).then_inc(s_xcA, 16)
            sync.wait_ge(v0, 1)
            sync.wait_ge(c0, 1)
            sync.dma_start(out0a_h[:], o0a[:]).then_inc(s_st0, 16)
            sync.wait_ge(s_st0, 16)

        @block.scalar
        def _(scalar):
            scalar.dma_start(xrowA[:], xrowA_h[:]).then_inc(s_xrA, 16)
            scalar.dma_start(xrowB[:], xrowB_h[:]).then_inc(s_xrB, 16)
            scalar.dma_start(wBsb[:], wB_h[:]).then_inc(s_wB, 16)
            scalar.wait_ge(t2, 1)
            scalar.copy(out=o0a[:, H : H + A], in_=psr[:]).then_inc(c0, 1)
            scalar.wait_ge(v1, 1)
            scalar.dma_start(out1_h[:], o1[:]).then_inc(s_st1, 16)
            scalar.wait_ge(s_st1, 16)

        @block.gpsimd
        def _(g):
            g.dma_start(wAsb[:], wA_h[:]).then_inc(s_wA, 16)
            g.dma_start(xcolB[:], xcolB_h[:]).then_inc(s_xcB, 16)

        @block.tensor
        def _(t):
            t.wait_ge(s_wA, 16)
            t.wait_ge(s_xcA, 16)
            t.matmul(ps0[:], lhsT=w_cs, rhs=xcA, start=True, stop=False)
            t.wait_ge(s_xrA, 16)
            t.matmul(ps0[:], lhsT=w_cw, rhs=xrowA[:, 0:H],
                     start=False, stop=True).then_inc(t0, 1)
            t.wait_ge(s_xcB, 16)
            t.matmul(ps1[:], lhsT=w_cs, rhs=xcolB[:], start=True, stop=False)
            t.wait_ge(s_xrB, 16)
            t.matmul(ps1[:], lhsT=w_cw, rhs=xrowB[:],
                     start=False, stop=True).then_inc(t1, 1)
            t.wait_ge(s_wB, 16)
            t.matmul(psr[:], lhsT=w_fcs, rhs=xcA_loc, start=True, stop=False)
            t.matmul(psr[:], lhsT=w_fcw, rhs=xrowA[:, H : H + A],
                     start=False, stop=False)
            t.matmul(psr[:], lhsT=w_rw, rhs=xrowA[:, H + A : H + 2 * A],
                     start=False, stop=False)
            t.matmul(psr[:], lhsT=w_rn, rhs=xcA_ns,
                     start=False, stop=True).then_inc(t2, 1)

        @block.vector
        def _(v):
            v.wait_ge(t0, 1)
            v.tensor_copy(out=o0a[:, 0:H], in_=ps0[:]).then_inc(v0, 1)
            v.wait_ge(t1, 1)
            v.tensor_copy(out=o1[:], in_=ps1[:]).then_inc(v1, 1)

    nc.compile()
    return nc


def _get_program():
    global _prog_cache
    if _prog_cache is None:
        _prog_cache = _build_program()
    return _prog_cache


def make_in_maps(inputs):
    inputs = {k: np.asarray(v) for k, v in inputs.items()}
    er = inputs["encoded_row"].astype(np.float32)          # [B, N, D]
    ec = inputs["encoded_col"].astype(np.float32)          # [B, S, D]
    clock = inputs["clock"].astype(np.float32)             # [B, 1]
    lpet = inputs["loc_process_end_time"].astype(np.float32)  # [B, P]
    W_dyn = inputs["W_dyn"].astype(np.float32)
    W_concat = inputs["W_concat"].astype(np.float32)
    W_robot = inputs["W_robot"].astype(np.float32)
    lhw = inputs["loc_hold_wafer"].astype(np.int64)        # [B, P]
    lst = inputs["loc_stage"].astype(np.int64)             # [B, P]
    loc = np.concatenate([inputs["robot_arm1_loc"], inputs["robot_arm2_loc"]],
                         axis=1).astype(np.int64)          # [B, 2]
    rec = np.concatenate([inputs["arm1_recipe"], inputs["arm2_recipe"]],
                         axis=1).astype(np.int64)          # [B, 2]
    nst = np.concatenate([inputs["arm1_next_stage"], inputs["arm2_next_stage"]],
                         axis=1).astype(np.int64)          # [B, 2]

    # pm ingredients, full batch
    rp = np.maximum(lpet - clock, 0.0) / NORM              # [B, P]
    wafer = np.where(
        (lhw >= 0)[:, :, None],
        np.take_along_axis(er, np.clip(lhw, 0, N - 1)[:, :, None], axis=1),
        0.0,
    )                                                      # [B, P, D]
    stage = np.take_along_axis(ec, (lst - 1)[:, :, None], axis=1)  # [B, P, D]

    # arm ingredients
    locv = (loc >= 1) & (loc <= P)                         # [B, 2]
    pidx = np.clip(loc - 1, 0, P - 1)
    armw = np.where(locv[:, :, None],
                    np.take_along_axis(wafer, pidx[:, :, None], axis=1), 0.0)
    arms = np.where(locv[:, :, None],
                    np.take_along_axis(stage, pidx[:, :, None], axis=1), 0.0)
    armr = np.where(locv, np.take_along_axis(rp, pidx, axis=1), 0.0)  # [B, 2]
    ind = (loc == P + 1).astype(np.float32)                # [B, 2]
    rrow = np.where(
        (rec >= 0)[:, :, None],
        np.take_along_axis(er, np.clip(rec, 0, N - 1)[:, :, None], axis=1),
        0.0,
    )                                                      # [B, 2, D]
    nsv = (nst >= 1) & (nst <= S)
    nrow = np.where(
        nsv[:, :, None],
        np.take_along_axis(ec, np.clip(nst - 1, 0, S - 1)[:, :, None], axis=1),
        0.0,
    )                                                      # [B, 2, D]

    # weights; the module is linear, so fold everything that is not a
    # gathered row into the inputs:
    #   arm head        -> fused weights W_cs@W_rl, W_cw@W_rl
    #   pm_dyn rank-1   -> min-norm z=[u;v]: W_cs.T u + W_cw.T v = v_dyn
    #   loc==P+1 ones   -> min-norm z2:      W_cs.T u2 + W_cw.T v2 = ones
    W_cs, W_cw, W_cd = W_concat[0:D], W_concat[D : 2 * D], W_concat[2 * D : 3 * D]
    W_rl, W_rw, W_rn = W_robot[0:D], W_robot[D : 2 * D], W_robot[2 * D : 3 * D]
    v_dyn = (W_dyn[0:1] @ W_cd).reshape(D)
    wA = np.concatenate([W_cs, W_cw], axis=1).astype(BF)   # [128, 2D]
    wB = np.ascontiguousarray(
        np.stack([W_rw, W_rn, W_cs @ W_rl, W_cw @ W_rl], axis=1)
    ).astype(BF)                                           # [128, 4, D]

    M = np.concatenate([W_cs.T, W_cw.T], axis=1).astype(np.float64)  # [D, 2D]
    z = np.linalg.lstsq(M, v_dyn.astype(np.float64), rcond=None)[0]
    u, v = z[:D].astype(np.float32), z[D:].astype(np.float32)
    z2 = np.linalg.lstsq(M, np.ones(D, np.float64), rcond=None)[0]
    u2, v2 = z2[:D].astype(np.float32), z2[D:].astype(np.float32)

    stage = stage + rp[:, :, None] * u                     # [B, P, D]
    wafer = wafer + rp[:, :, None] * v
    arms = arms + armr[:, :, None] * u + ind[:, :, None] * u2  # [B, 2, D]
    armw = armw + armr[:, :, None] * v + ind[:, :, None] * v2

    in_maps = []
    for c in range(NCORES):
        bs = slice(c * BL, (c + 1) * BL)
        xrow = np.concatenate(
            [wafer[bs].reshape(R, D), armw[bs].reshape(A, D),
             rrow[bs].reshape(A, D)], axis=0).T            # [D, R+2A]
        xcol = np.concatenate(
            [stage[bs].reshape(R, D), arms[bs].reshape(A, D),
             nrow[bs].reshape(A, D)], axis=0).T
        xrow = np.ascontiguousarray(xrow).astype(BF)
        xcol = np.ascontiguousarray(xcol).astype(BF)
        in_maps.append({
            "wA": wA,
            "xcolA": np.ascontiguousarray(np.concatenate(
                [xcol[:, 0:H], xcol[:, R : R + 2 * A]], axis=1)),
            "xcolB": np.ascontiguousarray(xcol[:, H:R]),
            "xrowA": np.ascontiguousarray(
                np.concatenate([xrow[:, 0:H], xrow[:, R : R + 2 * A]], axis=1)),
            "xrowB": np.ascontiguousarray(xrow[:, H:R]),
            "wB": wB,
        })
    return in_maps


def assemble_output(res):
    out = np.empty((B, P + 2, D), np.float32)
    for c in range(NCORES):
        bs = slice(c * BL, (c + 1) * BL)
        o0a = np.asarray(res[c]["out0a"])
        pmT = np.concatenate(
            [o0a[:, 0:H], np.asarray(res[c]["out1"])], axis=1
        ).astype(np.float32)                               # [D, R]
        out[bs, 0:P, :] = pmT.T.reshape(BL, P, D)
        armT = o0a[:, H : H + A].astype(np.float32)        # [D, A]
        out[bs, P:, :] = armT.T.reshape(BL, 2, D)
    return out


def kernel(**inputs):
    in_maps = make_in_maps(inputs)
    nc = _get_program()
    res = run_bass_kernel_spmd(nc, in_maps, list(range(NCORES))).results
    return assemble_output(res)


# revision 44
# speedup vs baseline: 1.0940x; 1.0940x over previous
"""Trainium2 Bass kernel for nn_CONCATNet_7447473291796 (gnn_message_passing).

Strategy (pure data parallelism, batch sharded 16 per core across 8 cores):
  The reference only ever *uses* ~66 of the 4096 wafer rows per batch. The
  host gathers exactly those rows (plus the stage / next-stage / arm rows)
  while sharding the batch, and hands each core dense, pre-transposed bf16
  tiles with the embed dim on partitions:

    xcolA [128, 832]   w_cs | w_cw | stage rows pm 0..511 | arm-loc | next-stage
    xrowA [128, 576]   wafer rows pm 0..511 | arm-loc | arm-recipe
    xcolB/xrowB [128, 512]  pm columns 512..1023

  The whole module is linear, so every non-gather term is folded on host:
  - robot-arm head -> fused weights (W_cs@W_rl, W_cw@W_rl)
  - the pm_dyn rank-1 term and the loc==P+1 ones-row -> a min-norm solve
    of [W_cs.T | W_cw.T] z = rhs (sigma_min ~0.23, so no noise blow-up)
    adds rp*u / rp*v (and ind*u2 / ind*v2) into the gathered rows.

  The device is then just 8 matmuls in a transposed layout out[d_out, rows]:
    pmT = W_cs.T @ xcol + W_cw.T @ xrow          (N=512, two tiles)
    armT = fused(W)s over the 64 arm columns      (N=32, four matmuls)

  Hand-scheduled raw bass (no TileContext): per-DMA semaphores, engines
  free-run with minimal waits.  bf16 in/out with fp32 PSUM accumulation
  keeps rel err ~4e-3 (gate is 2e-2).

All per-core variation lives in the DRAM inputs; the Bass program is
identical on every core.
"""

import numpy as np
import ml_dtypes

import concourse.bass as bass
import concourse.bacc as bacc
import concourse.mybir as mybir
from concourse.bass_utils import run_bass_kernel_spmd

B, N, S, P, D = 128, 4096, 32, 64, 128
NORM = 300.0
NCORES = 8
BL = B // NCORES          # local batches per core = 16
R = BL * P                # pm columns per core = 1024
A = 2 * BL                # arm columns per core = 32
H = R // 2                # pm columns per tile = 512
WA = 2 * D                # w_cs|w_cw packed at the head of xcolA

F32 = mybir.dt.float32
BF16 = mybir.dt.bfloat16
BF = ml_dtypes.bfloat16

_prog_cache = None


def _build_program():
    nc = bacc.Bacc("TRN2", target_bir_lowering=False, debug=False)

    xcolA_h = nc.declare_dram_parameter("xcolA", [128, WA + H + 2 * A], BF16,
                                        isOutput=False)
    xcolB_h = nc.declare_dram_parameter("xcolB", [128, H], BF16, isOutput=False)
    xrowA_h = nc.declare_dram_parameter("xrowA", [128, H + 2 * A], BF16, isOutput=False)
    xrowB_h = nc.declare_dram_parameter("xrowB", [128, H], BF16, isOutput=False)
    wB_h = nc.declare_dram_parameter("wB", [128, 4, D], BF16, isOutput=False)

    out0a_h = nc.declare_dram_parameter("out0a", [128, H + A], BF16, isOutput=True)
    out1_h = nc.declare_dram_parameter("out1", [128, H], BF16, isOutput=True)

    from contextlib import ExitStack
    with ExitStack() as stack:
        ec = stack.enter_context
        xcolA = ec(nc.sbuf_tensor([128, WA + H + 2 * A], BF16))
        xcolB = ec(nc.sbuf_tensor([128, H], BF16))
        xrowA = ec(nc.sbuf_tensor([128, H + 2 * A], BF16))
        xrowB = ec(nc.sbuf_tensor([128, H], BF16))
        wBsb = ec(nc.sbuf_tensor([128, 4, D], BF16))
        o0a = ec(nc.sbuf_tensor([128, H + A], BF16))
        o1 = ec(nc.sbuf_tensor([128, H], BF16))
        ps0 = ec(nc.psum_tensor([128, H], F32))
        ps1 = ec(nc.psum_tensor([128, H], F32))
        psr = ec(nc.psum_tensor([128, A], F32))
        s_xcA = ec(nc.semaphore("s_xcA"))
        s_xcB = ec(nc.semaphore("s_xcB"))
        s_xrA = ec(nc.semaphore("s_xrA"))
        s_xrB = ec(nc.semaphore("s_xrB"))
        s_wB = ec(nc.semaphore("s_wB"))
        s_st0 = ec(nc.semaphore("s_st0"))
        s_st1 = ec(nc.semaphore("s_st1"))
        t0 = ec(nc.semaphore("t0"))
        t1 = ec(nc.semaphore("t1"))
        t2 = ec(nc.semaphore("t2"))
        v0 = ec(nc.semaphore("v0"))
        v1 = ec(nc.semaphore("v1"))
        c0 = ec(nc.semaphore("c0"))
        block = ec(nc.Block())

        w_cs = xcolA[:, 0:D]
        w_cw = xcolA[:, D : 2 * D]
        w_rw = wBsb[:, 0, :]
        w_rn = wBsb[:, 1, :]
        w_fcs = wBsb[:, 2, :]    # W_cs @ W_rl
        w_fcw = wBsb[:, 3, :]    # W_cw @ W_rl
        xcA = xcolA[:, WA : WA + H]              # stage pm 0..511
        xcA_loc = xcolA[:, WA + H : WA + H + A]  # arm-loc stage (folded)
        xcA_ns = xcolA[:, WA + H + A : WA + H + 2 * A]

        @block.sync
        def _(sync):
            sync.dma_start(xcolA[:], xcolA_h[:]).then_inc(s_xcA, 16)
            sync.wait_ge(v0, 1)
            sync.wait_ge(c0, 1)
            sync.dma_start(out0a_h[:], o0a[:]).then_inc(s_st0, 16)
            sync.wait_ge(s_st0, 16)

        @block.scalar
        def _(scalar):
            scalar.dma_start(xrowA[:], xrowA_h[:]).then_inc(s_xrA, 16)
            scalar.dma_start(xrowB[:], xrowB_h[:]).then_inc(s_xrB, 16)
            scalar.dma_start(wBsb[:], wB_h[:]).then_inc(s_wB, 16)
            scalar.wait_ge(t2, 1)
            scalar.copy(out=o0a[:, H : H + A], in_=psr[:]).then_inc(c0, 1)
            scalar.wait_ge(v1, 1)
            scalar.dma_start(out1_h[:], o1[:]).then_inc(s_st1, 16)
            scalar.wait_ge(s_st1, 16)

        @block.gpsimd
        def _(g):
            g.dma_start(xcolB[:], xcolB_h[:]).then_inc(s_xcB, 16)

        @block.tensor
        def _(t):
            t.wait_ge(s_xcA, 16)
            t.matmul(ps0[:], lhsT=w_cs, rhs=xcA, start=True, stop=False)
            t.wait_ge(s_xrA, 16)
            t.matmul(ps0[:], lhsT=w_cw, rhs=xrowA[:, 0:H],
                     start=False, stop=True).then_inc(t0, 1)
            t.wait_ge(s_xcB, 16)
            t.matmul(ps1[:], lhsT=w_cs, rhs=xcolB[:], start=True, stop=False)
            t.wait_ge(s_xrB, 16)
            t.matmul(ps1[:], lhsT=w_cw, rhs=xrowB[:],
                     start=False, stop=True).then_inc(t1, 1)
            t.wait_ge(s_wB, 16)
            t.matmul(psr[:], lhsT=w_fcs, rhs=xcA_loc, start=True, stop=False)
            t.matmul(psr[:], lhsT=w_fcw, rhs=xrowA[:, H : H + A],
                     start=False, stop=False)
            t.matmul(psr[:], lhsT=w_rw, rhs=xrowA[:, H + A : H + 2 * A],
                     start=False, stop=False)
            t.matmul(psr[:], lhsT=w_rn, rhs=xcA_ns,
                     start=False, stop=True).then_inc(t2, 1)

        @block.vector
        def _(v):
            v.wait_ge(t0, 1)
            v.tensor_copy(out=o0a[:, 0:H], in_=ps0[:]).then_inc(v0, 1)
            v.wait_ge(t1, 1)
            v.tensor_copy(out=o1[:], in_=ps1[:]).then_inc(v1, 1)

    nc.compile()
    return nc


def _get_program():
    global _prog_cache
    if _prog_cache is None:
        _prog_cache = _build_program()
    return _prog_cache


def make_in_maps(inputs):
    inputs = {k: np.asarray(v) for k, v in inputs.items()}
    er = inputs["encoded_row"].astype(np.float32)          # [B, N, D]
    ec = inputs["encoded_col"].astype(np.float32)          # [B, S, D]
    clock = inputs["clock"].astype(np.float32)             # [B, 1]
    lpet = inputs["loc_process_end_time"].astype(np.float32)  # [B, P]
    W_dyn = inputs["W_dyn"].astype(np.float32)
    W_concat = inputs["W_concat"].astype(np.float32)
    W_robot = inputs["W_robot"].astype(np.float32)
    lhw = inputs["loc_hold_wafer"].astype(np.int64)        # [B, P]
    lst = inputs["loc_stage"].astype(np.int64)             # [B, P]
    loc = np.concatenate([inputs["robot_arm1_loc"], inputs["robot_arm2_loc"]],
                         axis=1).astype(np.int64)          # [B, 2]
    rec = np.concatenate([inputs["arm1_recipe"], inputs["arm2_recipe"]],
                         axis=1).astype(np.int64)          # [B, 2]
    nst = np.concatenate([inputs["arm1_next_stage"], inputs["arm2_next_stage"]],
                         axis=1).astype(np.int64)          # [B, 2]

    # pm ingredients, full batch
    rp = np.maximum(lpet - clock, 0.0) / NORM              # [B, P]
    wafer = np.where(
        (lhw >= 0)[:, :, None],
        np.take_along_axis(er, np.clip(lhw, 0, N - 1)[:, :, None], axis=1),
        0.0,
    )                                                      # [B, P, D]
    stage = np.take_along_axis(ec, (lst - 1)[:, :, None], axis=1)  # [B, P, D]

    # arm ingredients
    locv = (loc >= 1) & (loc <= P)                         # [B, 2]
    pidx = np.clip(loc - 1, 0, P - 1)
    armw = np.where(locv[:, :, None],
                    np.take_along_axis(wafer, pidx[:, :, None], axis=1), 0.0)
    arms = np.where(locv[:, :, None],
                    np.take_along_axis(stage, pidx[:, :, None], axis=1), 0.0)
    armr = np.where(locv, np.take_along_axis(rp, pidx, axis=1), 0.0)  # [B, 2]
    ind = (loc == P + 1).astype(np.float32)                # [B, 2]
    rrow = np.where(
        (rec >= 0)[:, :, None],
        np.take_along_axis(er, np.clip(rec, 0, N - 1)[:, :, None], axis=1),
        0.0,
    )                                                      # [B, 2, D]
    nsv = (nst >= 1) & (nst <= S)
    nrow = np.where(
        nsv[:, :, None],
        np.take_along_axis(ec, np.clip(nst - 1, 0, S - 1)[:, :, None], axis=1),
        0.0,
    )                                                      # [B, 2, D]

    # weights; the module is linear, so fold everything that is not a
    # gathered row into the inputs:
    #   arm head        -> fused weights W_cs@W_rl, W_cw@W_rl
    #   pm_dyn rank-1   -> min-norm z=[u;v]: W_cs.T u + W_cw.T v = v_dyn
    #   loc==P+1 ones   -> min-norm z2:      W_cs.T u2 + W_cw.T v2 = ones
    W_cs, W_cw, W_cd = W_concat[0:D], W_concat[D : 2 * D], W_concat[2 * D : 3 * D]
    W_rl, W_rw, W_rn = W_robot[0:D], W_robot[D : 2 * D], W_robot[2 * D : 3 * D]
    v_dyn = (W_dyn[0:1] @ W_cd).reshape(D)
    wA = np.concatenate([W_cs, W_cw], axis=1).astype(BF)   # [128, 2D]
    wB = np.ascontiguousarray(
        np.stack([W_rw, W_rn, W_cs @ W_rl, W_cw @ W_rl], axis=1)
    ).astype(BF)                                           # [128, 4, D]

    M = np.concatenate([W_cs.T, W_cw.T], axis=1).astype(np.float64)  # [D, 2D]
    z = np.linalg.lstsq(M, v_dyn.astype(np.float64), rcond=None)[0]
    u, v = z[:D].astype(np.float32), z[D:].astype(np.float32)
    z2 = np.linalg.lstsq(M, np.ones(D, np.float64), rcond=None)[0]
    u2, v2 = z2[:D].astype(np.float32), z2[D:].astype(np.float32)

    stage = stage + rp[:, :, None] * u                     # [B, P, D]
    wafer = wafer + rp[:, :, None] * v
    arms = arms + armr[:, :, None] * u + ind[:, :, None] * u2  # [B, 2, D]
    armw = armw + armr[:, :, None] * v + ind[:, :, None] * v2

    in_maps = []
    for c in range(NCORES):
        bs = slice(c * BL, (c + 1) * BL)
        xrow = np.concatenate(
            [wafer[bs].reshape(R, D), armw[bs].reshape(A, D),
             rrow[bs].reshape(A, D)], axis=0).T            # [D, R+2A]
        xcol = np.concatenate(
            [stage[bs].reshape(R, D), arms[bs].reshape(A, D),
             nrow[bs].reshape(A, D)], axis=0).T
        xrow = np.ascontiguousarray(xrow).astype(BF)
        xcol = np.ascontiguousarray(xcol).astype(BF)
        in_maps.append({
            "xcolA": np.ascontiguousarray(np.concatenate(
                [wA, xcol[:, 0:H], xcol[:, R : R + 2 * A]], axis=1)),
            "xcolB": np.ascontiguousarray(xcol[:, H:R]),
            "xrowA": np.ascontiguousarray(
                np.concatenate([xrow[:, 0:H], xrow[:, R : R + 2 * A]], axis=1)),
            "xrowB": np.ascontiguousarray(xrow[:, H:R]),
            "wB": wB,
        })
    return in_maps


def assemble_output(res):
    out = np.empty((B, P + 2, D), np.float32)
    for c in range(NCORES):
        bs = slice(c * BL, (c + 1) * BL)
        o0a = np.asarray(res[c]["out0a"])
        pmT = np.concatenate(
            [o0a[:, 0:H], np.asarray(res[c]["out1"])], axis=1
        ).astype(np.float32)                               # [D, R]
        out[bs, 0:P, :] = pmT.T.reshape(BL, P, D)
        armT = o0a[:, H : H + A].astype(np.float32)        # [D, A]
        out[bs, P:, :] = armT.T.reshape(BL, 2, D)
    return out


def kernel(**inputs):
    in_maps = make_in_maps(inputs)
    nc = _get_program()
    res = run_bass_kernel_spmd(nc, in_maps, list(range(NCORES))).results
    return assemble_output(res)
